# revision 18
# baseline (speedup 1.0000x reference)
"""GQA causal attention (batch 2, seq 2048, hidden 2048, 16 Q heads / 4 KV heads,
head_dim 128) on 8 trn2 NeuronCores.

Sharding: data-parallel over batch (2) x tensor-parallel over KV-head groups (4).
Core c = b*4 + g handles batch b, Q heads [4g, 4g+4), KV head g, and produces a
partial output x_attn @ wo[512g:512g+512, :]; the host sums the 4 partials per
batch (the unshard step for row-sharded o_proj input).

On-chip layout strategy: everything lives transposed so no large transposes are
needed. Host passes x^T [hidden, seq]. Projections compute qT/kT/vT [d, seq]
(weights stationary). V natural [k, d] is produced by SBUF->SBUF DMA-transpose
(xbar), not the PE. Scores are computed transposed [k, q] so that:
  - exp runs on ScalarE psum->sbuf,
  - PV^T = v[k,d]^T-contraction consumes probsT [k, q] directly,
  - PV^T output [d, q] is exactly the lhsT layout o_proj needs.
Softmax denominators: probsT chunks are tree-summed on DVE (plus staggered
width-restricted adds for the diagonal chunks) into one [128,512] tile per
(q-block, head); a single ones-column matmul then reduces over partitions and
broadcasts the row-sums to all 128 partitions in one shot.
Causality: score blocks strictly above the diagonal are skipped entirely; the
4 diagonal-crossing 128x512 chunks are computed width-restricted and stored
PACKED (only the valid columns), so one exp covers several chunks and only one
[128,128] lower-triangle 0/1 mask (identical for every chunk) is multiplied in
after exp (exp(s)*m == exp(s + log m) matches the reference's additive -1e9).
"""

import sys

sys.path.insert(0, "/opt/trn_rl_repo")

from contextlib import ExitStack

import ml_dtypes
import numpy as np

import concourse.bass as bass  # noqa: F401  (import keeps bass registered)
import concourse.mybir as mybir
import concourse.tile as tile
from concourse import bacc
from concourse.bass_utils import run_bass_kernel_spmd

BF16 = mybir.dt.bfloat16
F32 = mybir.dt.float32
NPBF16 = ml_dtypes.bfloat16

B = 2
S = 2048
H = 2048
D = 128
N_HEADS = 16
N_KV = 4
QC = 512  # q columns per core (4 heads x 128)
NHC = H // 128  # 16 hidden chunks
NRB = S // 512  # 4 row blocks of 512
NQC = QC // 128  # 4 head chunks per core
NKC = S // 128  # 16 key chunks of 128
SCALE = 1.0 / float(np.sqrt(D))

# Packed column offsets of the 4 diagonal chunks (widths 512,384,256,128).
# X psum tile holds d0,d1; Y psum tile holds d2,d3.
DIAG_W = [512, 384, 256, 128]
DIAG_PK = [0, 512, 896, 1152]  # offsets in the packed pTd sbuf tile
DIAG_PSX = [0, 512]  # offsets of d0,d1 inside the X psum tile
DIAG_PSY = [0, 256]  # offsets of d2,d3 inside the Y psum tile

MASK_ON_GPSIMD = True

_NC = None


def _build_nc():
    nc = bacc.Bacc("TRN2", target_bir_lowering=False, debug=False, num_devices=8)

    xT = nc.dram_tensor("xT", [H, S], BF16, kind="ExternalInput")
    wq = nc.dram_tensor("wq", [H, QC], BF16, kind="ExternalInput")
    wk = nc.dram_tensor("wk", [H, D], BF16, kind="ExternalInput")
    wv = nc.dram_tensor("wv", [H, D], BF16, kind="ExternalInput")
    wo = nc.dram_tensor("wo", [QC, H], BF16, kind="ExternalInput")
    bq = nc.dram_tensor("bq", [QC, 1], F32, kind="ExternalInput")
    bk = nc.dram_tensor("bk", [D, 1], F32, kind="ExternalInput")
    bv = nc.dram_tensor("bv", [D, 1], F32, kind="ExternalInput")
    tri = nc.dram_tensor("tri", [128, 128], BF16, kind="ExternalInput")
    out = nc.dram_tensor("out", [S, H], BF16, kind="ExternalOutput")

    with tile.TileContext(nc) as tc, ExitStack() as ctx:
        persist = ctx.enter_context(tc.tile_pool(name="persist", bufs=1))

        # qT / AT as separate per-512-block tiles: tile-granular dependency
        # tracking would otherwise serialize attention reads of early blocks
        # behind later projection / AT writes to the same big tile.
        qT = [
            [persist.tile([128, 512], BF16, tag=f"qT{i}_{rb}", name=f"qT{i}_{rb}") for rb in range(NRB)]
            for i in range(NQC)
        ]
        kT = persist.tile([128, S], BF16, tag="kT", name="kT")
        vT = persist.tile([128, S], BF16, tag="vT", name="vT")
        v_sb = persist.tile([128, NKC, D], BF16, tag="v", name="v_sb")
        AT = [
            [persist.tile([128, 512], BF16, tag=f"AT{i}_{g}", name=f"AT{i}_{g}") for g in range(NRB)]
            for i in range(NQC)
        ]
        tri_sb = persist.tile([128, 128], BF16, tag="tri", name="tri_sb")
        bq_sb = persist.tile([128, NQC], F32, tag="bq", name="bq_sb")
        bk_sb = persist.tile([128, 1], F32, tag="bk", name="bk_sb")
        bv_sb = persist.tile([128, 1], F32, tag="bv", name="bv_sb")
        ones_mat = persist.tile([128, 128], BF16, tag="onesm", name="ones_mat")

        nc.vector.memset(ones_mat, 1.0)
        nc.gpsimd.dma_start(out=tri_sb, in_=tri[:, :])
        # pre-warm the GpSimd tensor-op ucode library (LOAD_LIB) so the first
        # real mask multiply doesn't pay the swap mid-kernel (1*1==1 is a noop)
        nc.gpsimd.tensor_mul(ones_mat[:, 0:1], ones_mat[:, 0:1], ones_mat[:, 0:1])
        for qc in range(NQC):
            nc.gpsimd.dma_start(
                out=bq_sb[:, qc : qc + 1], in_=bq[qc * 128 : (qc + 1) * 128, :]
            )
        nc.gpsimd.dma_start(out=bk_sb, in_=bk[:, :])
        nc.gpsimd.dma_start(out=bv_sb, in_=bv[:, :])

        # ---------------- Phase 1: projections ----------------
        # p1a (xt cols 0:1024, wk, wv) closes after the K/V wave; p1b (xt cols
        # 1024:2048, wq) must outlive phase 1: the last two Q-projection psum
        # groups are injected into the first attention q-group's pipeline.
        # p1b under p1a on the "right" SBUF stack: pools release LIFO per side,
        # and p1a is closed first (after K/V), p1b later (after q-group 1).
        p1a_stack = ExitStack()
        p1b_stack = ExitStack()
        pp1_stack = ExitStack()
        p1b = p1b_stack.enter_context(tc.tile_pool(name="p1b", bufs=1, side="right"))
        p1a = p1a_stack.enter_context(tc.tile_pool(name="p1a", bufs=1, side="right"))
        pp1 = pp1_stack.enter_context(tc.tile_pool(name="psum1", bufs=8, space="PSUM"))
        p2 = ctx.enter_context(tc.tile_pool(name="p2", bufs=1))
        if True:
            # HAM warmup: ~3.4us of dummy matmuls on already-available data so
            # the PE clock gate opens (1.2 -> 2.4 GHz) before the first real
            # wave instead of ~4us into it.  They finish before the first xT
            # chunk lands, so they cost no wall-clock.
            warm = pp1.tile([128, 512], F32, tag="pp", name="warm")
            for _ in range(30):
                nc.tensor.matmul(
                    warm[:, :128], ones_mat, ones_mat, start=True, stop=True
                )

            xtA = [p1a.tile([128, 1024], BF16, tag=f"xa{hc}", name=f"xa{hc}") for hc in range(NHC)]
            xtB = [p1b.tile([128, 1024], BF16, tag=f"xb{hc}", name=f"xb{hc}") for hc in range(NHC)]
            wq_sb = [p1b.tile([128, QC], BF16, tag=f"wq{hc}", name=f"wq{hc}") for hc in range(NHC)]
            wk_sb = [p1a.tile([128, D], BF16, tag=f"wk{hc}", name=f"wk{hc}") for hc in range(NHC)]
            wv_sb = [p1a.tile([128, D], BF16, tag=f"wv{hc}", name=f"wv{hc}") for hc in range(NHC)]
            wo_sb = [p2.tile([128, H], BF16, tag=f"wo{qc}", name=f"wo{qc}") for qc in range(NQC)]

            # DMAs in consumption order, spread across engine queues for
            # bandwidth (each engine issues to its own HWDGE queue).
            # wk/wv are not needed until the K/V wave (~25us after the Q wave
            # starts); queue them BEHIND the xtA+wq stream the Q wave consumes
            # immediately, so the PE never catches up to the x DMA stream.
            for hc in range(NHC):
                if hc == 0:
                    nc.sync.dma_start(out=xtA[0][:, :512], in_=xT[:128, :512])
                    nc.sync.dma_start(out=xtA[0][:, 512:], in_=xT[:128, 512:1024])
                else:
                    nc.sync.dma_start(
                        out=xtA[hc], in_=xT[hc * 128 : (hc + 1) * 128, :1024]
                    )
                nc.scalar.dma_start(out=wq_sb[hc], in_=wq[hc * 128 : (hc + 1) * 128, :])
            for hc in range(NHC):
                nc.gpsimd.dma_start(out=wk_sb[hc], in_=wk[hc * 128 : (hc + 1) * 128, :])
                nc.gpsimd.dma_start(out=wv_sb[hc], in_=wv[hc * 128 : (hc + 1) * 128, :])
            for hc in range(NHC):
                nc.sync.dma_start(
                    out=xtB[hc], in_=xT[hc * 128 : (hc + 1) * 128, 1024:]
                )
            # wo is not needed until the first injected o_proj (~halfway in).
            # Queue it on the SAME HWDGE ring as x, BEHIND all x chunks: queues
            # drain round-robin per SDMA engine, so a separate ring would steal
            # HBM bandwidth from x exactly when Q wave 1 is consuming it.
            for qc in range(NQC):
                nc.sync.dma_start(out=wo_sb[qc], in_=wo[qc * 128 : (qc + 1) * 128, :])

            def xt_rb(hc, rb):
                t = xtA[hc] if rb < 2 else xtB[hc]
                return t[:, (rb % 2) * 512 : (rb % 2) * 512 + 512]

            def q_wave(groups, pool):
                qps = {
                    (qc, rb): pool.tile([128, 512], F32, tag="pp" if pool is pp1 else "oi", name="qp")
                    for qc, rb in groups
                }
                for hc in range(NHC):
                    st = hc == 0
                    sp = hc == NHC - 1
                    for qc, rb in groups:
                        lhsT = wq_sb[hc][:, qc * 128 : (qc + 1) * 128]
                        nc.tensor.matmul(
                            qps[qc, rb], lhsT, xt_rb(hc, rb), start=st, stop=sp
                        )
                for qc, rb in groups:
                    nc.vector.tensor_scalar_add(
                        qT[qc][rb], qps[qc, rb], bq_sb[:, qc : qc + 1]
                    )

            # Q wave 1 first (dense PE work absorbs the xT DMA stream).
            q_wave([(qc, rb) for qc in range(NQC) for rb in (0, 1)], pp1)

            # K/V wave, row-block-major: vT row blocks finish early so their
            # xbar DMA-transposes (v natural chunks; all on one engine --
            # concurrent xbar transposes from two HWDGE rings corrupt data)
            # complete long before attention needs them.
            for rb in range(NRB):
                sl = slice(rb * 512, (rb + 1) * 512)
                kps = pp1.tile([128, 512], F32, tag="pp", name="kp")
                vps = pp1.tile([128, 512], F32, tag="pp", name="vp")
                for hc in range(NHC):
                    st = hc == 0
                    sp = hc == NHC - 1
                    rhs = xt_rb(hc, rb)
                    nc.tensor.matmul(kps, wk_sb[hc], rhs, start=st, stop=sp)
                    nc.tensor.matmul(vps, wv_sb[hc], rhs, start=st, stop=sp)
                nc.vector.tensor_scalar_add(kT[:, sl], kps, bk_sb)
                nc.vector.tensor_scalar_add(vT[:, sl], vps, bv_sb)
                for kc in range(4 * rb, 4 * rb + 4):
                    nc.sync.dma_start_transpose(
                        v_sb[:, kc, :], vT[:, kc * 128 : (kc + 1) * 128]
                    )
            p1a_stack.close()

            # Q wave 2 minus the (qc=2,3 / rb=2,3) groups (those are deferred
            # into qg 1's attention pipeline, which otherwise has PE bubbles:
            # ScalarE exps pace it and little o_proj work exists yet), in
            # sub-waves so the last pp1 drain (whose completion gates
            # attention's first psum writes through bank reuse) is only one
            # bias-add deep.
            q_wave([(0, 2), (0, 3)], pp1)
            q_wave([(1, 2)], pp1)
            q_wave([(1, 3)], pp1)
            pp1_stack.close()

        # ---------------- Phase 2: attention ----------------
        with tc.tile_pool(name="p2b", bufs=2) as p2b:

            att_pools = ExitStack()
            # ps_oi FIRST: it gets the lowest psum bytes (earliest-drained pp1
            # banks), so the pre-popped deferred Q-projection mms can start
            # while the last q-wave bias-adds still drain the high banks.
            # 2 banks for injected o_proj so consecutive groups don't serialize
            # against the DVE psum->sbuf copy.
            ps_oi = att_pools.enter_context(tc.tile_pool(name="ps_oi", bufs=2, space="PSUM"))
            ps_s = att_pools.enter_context(tc.tile_pool(name="ps_s", bufs=2, space="PSUM"))
            ps_pv = att_pools.enter_context(tc.tile_pool(name="ps_pv", bufs=1, space="PSUM"))
            ps_den = att_pools.enter_context(tc.tile_pool(name="ps_den", bufs=1, space="PSUM"))

            def oproj_inject_ops(rc):
                """Single-bank o_proj ops for one row chunk: 4 groups of
                (4 matmuls + DVE cast + DMA), injected into the next q group's
                attention pipeline."""
                ops = []
                rsl = slice(rc * 128, (rc + 1) * 128)
                state = {}
                for oc in range(NRB):
                    osl = slice(oc * 512, (oc + 1) * 512)
                    for qc in range(NQC):
                        def mk(qc=qc, oc=oc, osl=osl, rsl=rsl):
                            if qc == 0:
                                state[oc] = ps_oi.tile(
                                    [128, 512], F32, tag="oi", name="oips"
                                )
                            nc.tensor.matmul(
                                state[oc],
                                AT[qc][rc // 4][:, (rc % 4) * 128 : (rc % 4) * 128 + 128],
                                wo_sb[qc][:, osl],
                                start=(qc == 0),
                                stop=(qc == NQC - 1),
                            )
                        ops.append(mk)
                    def mkcp(oc=oc, osl=osl, rsl=rsl):
                        # stage the whole 128-row chunk in ONE [128,2048] tile
                        # and ship it with ONE dma: out[rsl, :] is contiguous
                        # in HBM (row stride == chunk row size), so the single
                        # descriptor streams 512KB, and the sync sequencer
                        # writes 1 descriptor per chunk instead of 4.
                        if oc == 0:
                            state["ost"] = p2b.tile(
                                [128, 2048], BF16, tag="osti", name="ostagei", bufs=2
                            )
                        # all on DVE: a copy in ScalarE's queue delays every
                        # exp behind it, and exps pace the attention pipeline
                        nc.vector.tensor_copy(state["ost"][:, osl], state.pop(oc))
                        if oc == NRB - 1:
                            nc.sync.dma_start(out=out[rsl, :], in_=state.pop("ost"))
                    ops.append(mkcp)
                return ops

            def q2c_inject_ops():
                """The deferred (qc=2,3 / rb=2,3) Q-projection groups, run
                inside qg 1's attention pipeline out of the ps_oi banks (which
                are idle until the first o_proj injection a whole q-group
                later).  Ordered earliest-needed first: qT[2][2] / qT[3][2]
                are consumed by qg 2's h=2,3; the rb=3 groups only by qg 3."""
                ops = []
                state = {}
                for qc, rb in ((2, 2), (3, 2), (2, 3), (3, 3)):
                    for hc in range(NHC):
                        def mk(qc=qc, rb=rb, hc=hc):
                            if hc == 0:
                                state[qc, rb] = ps_oi.tile(
                                    [128, 512], F32, tag="oi", name="q2cps"
                                )
                            nc.tensor.matmul(
                                state[qc, rb],
                                wq_sb[hc][:, qc * 128 : (qc + 1) * 128],
                                xt_rb(hc, rb),
                                start=(hc == 0),
                                stop=(hc == NHC - 1),
                            )
                        ops.append(mk)
                    def mkadd(qc=qc, rb=rb):
                        nc.vector.tensor_scalar_add(
                            qT[qc][rb], state.pop((qc, rb)), bq_sb[:, qc : qc + 1]
                        )
                    ops.append(mkadd)
                return ops

            # q groups ordered so the small qg 0 runs LAST: its pipeline
            # bubbles (only 2 units/head) are filled by injected o_proj work,
            # which only becomes available after other groups have finished.
            # qg 1 (first, nothing else available) absorbs the deferred qc=3
            # Q-projection groups instead.
            QG_ORDER = [1, 2, 3, 0]
            pending = q2c_inject_ops()
            for qg in QG_ORDER:
                nod = 4 * qg  # number of off-diagonal (full) chunks
                qsl = slice(qg * 512, (qg + 1) * 512)
                for h in range(NQC):
                    # off-diagonal probs, full width
                    pT = p2b.tile([128, 12, 512], BF16, tag="pT", name="pT")
                    # packed diagonal probs (widths 512,384,256,128)
                    pTd = p2b.tile([128, 1280], BF16, tag="pTd", name="pTd")
                    pvps = ps_pv.tile([128, 512], F32, tag="pv", name="pvps")
                    denps = ps_den.tile([128, 512], F32, tag="den", name="denps")

                    # tq[lvl] tiles for the DVE denominator tree
                    tq_lvl1 = []  # pair sums of off-diag chunks
                    tq_final = None

                    # Software pipeline over "units": off-diagonal chunks in
                    # pairs sharing one [128,1024] psum tile and ONE exp
                    # (halves ScalarE's per-instruction overhead); the 4
                    # diagonal chunks as two packed units X=(d0,d1), Y=(d2,d3).
                    # pv MMs lag LAGU units.
                    # diagonal units FIRST: their exps then sit at the head of
                    # ScalarE's queue, so the GpSimd mask muls are long done
                    # before the lagged pv MMs need them (ScalarE runs ~1.1us
                    # per pair exp vs ~0.6us of PE per pair unit and falls
                    # behind over a head; the diagonal must not queue last).
                    units = [("X",), ("Y",)]
                    units += [("p", 2 * u, 2 * u + 1) for u in range(nod // 2)]
                    nu = len(units)
                    LAGU = 5  # pv lag: gives GpSimd mask muls slack after exp

                    # this head's share of the injected o_proj ops; a few are
                    # held back to pad the PE queue around the DVE-dependent
                    # denominator chain at the end of the head.
                    quota = -(-len(pending) // (NQC - h)) if pending else 0
                    mine = [pending.pop(0) for _ in range(quota) if pending]
                    reserve = min(8, len(mine))
                    body = len(mine) - reserve

                    def emit_pv(unit):
                        if unit[0] == "p":
                            for j in unit[1:]:
                                nc.tensor.matmul(
                                    pvps,
                                    v_sb[:, j, :],
                                    pT[:, j, :],
                                    start=False,
                                    stop=(j == nod - 1),
                                )
                            a, b = unit[1], unit[2]
                            tq = p2b.tile([128, 512], BF16, tag="tq", name="tq", bufs=8)
                            nc.vector.tensor_add(tq, pT[:, a, :], pT[:, b, :])
                            tq_lvl1.append(tq)
                            if len(tq_lvl1) == 1:
                                # fold the packed diagonal chunks into the
                                # first tree node NOW (pTd is long since
                                # written): the post-units DVE tail is then
                                # just the tree, not tree + 4 staggered adds,
                                # so the den matmul stops stalling on DVE at
                                # the last head of each q-group.
                                for vv in range(4):
                                    o = 128 * vv
                                    nc.vector.tensor_add(
                                        tq[:, o:512],
                                        tq[:, o:512],
                                        pTd[:, DIAG_PK[vv] : DIAG_PK[vv] + DIAG_W[vv]],
                                    )
                        elif unit[0] == "X":
                            for vv in range(2):
                                o = 128 * vv
                                nc.tensor.matmul(
                                    pvps[:, o:],
                                    v_sb[:, nod + vv, :],
                                    pTd[:, DIAG_PK[vv] : DIAG_PK[vv] + DIAG_W[vv]],
                                    start=(vv == 0),
                                    stop=False,
                                )
                        else:  # Y
                            for vv in range(2, 4):
                                o = 128 * vv
                                nc.tensor.matmul(
                                    pvps[:, o:],
                                    v_sb[:, nod + vv, :],
                                    pTd[:, DIAG_PK[vv] : DIAG_PK[vv] + DIAG_W[vv]],
                                    start=False,
                                    stop=(nod == 0 and vv == 3),
                                )

                    for ui in range(nu + LAGU):
                        if ui == 0 and h == 0 and body > 0:
                            # q-group starts: the first sps write stalls on a
                            # psum drain from the previous phase/group (pp1
                            # bias-adds at attention start; the previous
                            # group's exp backlog later).  Injected ops write
                            # other banks -- run a few into that gap first.
                            npop = min(body, 4)
                            for _ in range(npop):
                                mine.pop(0)()
                            body -= npop
                        if ui < nu:
                            unit = units[ui]
                            sps = ps_s.tile([128, 1024], F32, tag="s", name="sps")
                            if unit[0] == "p":
                                a, b = unit[1], unit[2]
                                for half, j in enumerate(unit[1:]):
                                    nc.tensor.matmul(
                                        sps[:, half * 512 : half * 512 + 512],
                                        kT[:, j * 128 : (j + 1) * 128],
                                        qT[h][qg],
                                        start=True,
                                        stop=True,
                                    )
                                nc.scalar.activation(
                                    pT[:, a : a + 2, :],
                                    sps[:, :],
                                    mybir.ActivationFunctionType.Exp,
                                    scale=SCALE,
                                )
                            elif unit[0] == "X":
                                for vv in range(2):
                                    kc = nod + vv
                                    o = 128 * vv
                                    nc.tensor.matmul(
                                        sps[:, DIAG_PSX[vv] : DIAG_PSX[vv] + DIAG_W[vv]],
                                        kT[:, kc * 128 : (kc + 1) * 128],
                                        qT[h][qg][:, o:],
                                        start=True,
                                        stop=True,
                                    )
                                nc.scalar.activation(
                                    pTd[:, 0:896],
                                    sps[:, 0:896],
                                    mybir.ActivationFunctionType.Exp,
                                    scale=SCALE,
                                )
                                eng = nc.gpsimd if MASK_ON_GPSIMD else nc.vector
                                for vv in range(2):
                                    pk = DIAG_PK[vv]
                                    eng.tensor_mul(
                                        pTd[:, pk : pk + 128],
                                        pTd[:, pk : pk + 128],
                                        tri_sb,
                                    )
                            else:  # Y
                                for vv in range(2, 4):
                                    kc = nod + vv
                                    o = 128 * vv
                                    nc.tensor.matmul(
                                        sps[:, DIAG_PSY[vv - 2] : DIAG_PSY[vv - 2] + DIAG_W[vv]],
                                        kT[:, kc * 128 : (kc + 1) * 128],
                                        qT[h][qg][:, o:],
                                        start=True,
                                        stop=True,
                                    )
                                nc.scalar.activation(
                                    pTd[:, 896:1280],
                                    sps[:, 0:384],
                                    mybir.ActivationFunctionType.Exp,
                                    scale=SCALE,
                                )
                                eng = nc.gpsimd if MASK_ON_GPSIMD else nc.vector
                                for vv in range(2, 4):
                                    pk = DIAG_PK[vv]
                                    eng.tensor_mul(
                                        pTd[:, pk : pk + 128],
                                        pTd[:, pk : pk + 128],
                                        tri_sb,
                                    )
                        if body > 0:
                            steps_left = nu + LAGU - ui
                            npop = min(body, -(-body // max(1, steps_left)))
                            for _ in range(npop):
                                mine.pop(0)()
                            body -= npop
                        uj = ui - LAGU
                        if 0 <= uj < nu:
                            emit_pv(units[uj])

                    # Denominator: DVE tree over off-diag pair sums, then
                    # staggered adds of the packed diagonal chunks, then ONE
                    # ones-matmul partition-reduce+broadcast.  The reserved
                    # o_proj ops pad the PE queue while DVE works.
                    if nod == 0:
                        while len(mine) > 2:
                            mine.pop(0)()
                        # qg 0: reduce the packed diagonal chunks directly.
                        for vv in range(4):
                            nc.tensor.matmul(
                                denps[:, 128 * vv : 512],
                                ones_mat,
                                pTd[:, DIAG_PK[vv] : DIAG_PK[vv] + DIAG_W[vv]],
                                start=(vv == 0),
                                stop=(vv == 3),
                            )
                    else:
                        lvl = tq_lvl1
                        while len(lvl) > 1:
                            nxt = []
                            for i in range(0, len(lvl) - 1, 2):
                                t = p2b.tile([128, 512], BF16, tag="tq", name="tqn", bufs=8)
                                nc.vector.tensor_add(t, lvl[i], lvl[i + 1])
                                nxt.append(t)
                            if len(lvl) % 2 == 1:
                                nxt.append(lvl[-1])
                            lvl = nxt
                        tq_final = lvl[0]
                        while len(mine) > 2:
                            mine.pop(0)()
                        nc.tensor.matmul(
                            denps, ones_mat, tq_final, start=True, stop=True
                        )

                    for f in mine:
                        f()
                    recip_sb = p2b.tile([128, 512], F32, tag="recip", name="recip_sb")
                    nc.vector.reciprocal_approx_fast(recip_sb, denps)
                    nc.vector.tensor_mul(AT[h][qg], pvps, recip_sb)
                for f in pending:
                    f()
                pending = []
                if qg == 1:
                    p1b_stack.close()
                if qg != QG_ORDER[-1]:
                    for rc in range(4 * qg, 4 * qg + 4):
                        pending.extend(oproj_inject_ops(rc))

            # ---------------- Phase 3: o_proj tail (last q group's rows) -----
            # No fresh PSUM pool: a new pool's first write joins on the FULL
            # attention-pool drain (1.8us: the last head's AT-mul).  Instead
            # scavenge 8 groups from the still-open pools, allocating the
            # late-draining pv/den banks behind a lead of runnable matmuls.
            # rc 0/1 first run only qc=0..2 (24 mms): runnable as soon as
            # AT[0..2][0] exist, padding the PE while the last head's
            # den/recip/AT-mul DVE chain produces AT[3][0].  Each rc's qc=3
            # pass completes its psum group and is followed immediately by its
            # copies+DMAs, so they drain under the next rc's matmuls; the
            # post-last-matmul tail is just one rc's copies+DMAs (~1.5us).
            # Final DMAs go on sync/scalar only: their end-of-kernel queue
            # drains are ~0.2us, vs 4.7us for gpsimd (hidden mid-kernel).
            aps = {}

            def p3_alloc(rc):
                if rc % 2 == 0:
                    sA = ps_s.tile([128, 1024], F32, tag="s", name="p3a")
                    sB = ps_s.tile([128, 1024], F32, tag="s", name="p3b")
                    aps[rc] = [sA[:, :512], sA[:, 512:], sB[:, :512], sB[:, 512:]]
                else:
                    aps[rc] = [
                        ps_oi.tile([128, 512], F32, tag="oi", name="p3o1"),
                        ps_oi.tile([128, 512], F32, tag="oi", name="p3o2"),
                        ps_pv.tile([128, 512], F32, tag="pv", name="p3o3"),
                        ps_den.tile([128, 512], F32, tag="den", name="p3o4"),
                    ]

            def p3_mms(rc, qcs):
                for qc in qcs:
                    for oc in range(NRB):
                        nc.tensor.matmul(
                            aps[rc][oc],
                            AT[qc][0][:, rc * 128 : rc * 128 + 128],
                            wo_sb[qc][:, oc * 512 : (oc + 1) * 512],
                            start=(qc == 0),
                            stop=(qc == NQC - 1),
                        )

            def p3_fin(rc):
                # final qc=3 accumulation interleaved with copies into two
                # [128,1024] staging tiles (scalar/vector in parallel per
                # half), each half shipped as soon as its two copies land.
                # Separate tiles per half: one [128,2048] tile would WAR-
                # serialize the second half's copies behind the first half's
                # dma read (tile-granular dependency tracking).
                rsl = slice(rc * 128, (rc + 1) * 128)
                for half in range(2):
                    ost = p2b.tile([128, 1024], BF16, tag="ost5", name="ost5", bufs=4)
                    for sub in range(2):
                        oc = 2 * half + sub
                        nc.tensor.matmul(
                            aps[rc][oc],
                            AT[3][0][:, rc * 128 : rc * 128 + 128],
                            wo_sb[3][:, oc * 512 : (oc + 1) * 512],
                            start=False,
                            stop=True,
                        )
                        if sub == 0:
                            nc.scalar.copy(ost[:, :512], aps[rc][oc])
                        else:
                            nc.vector.tensor_copy(ost[:, 512:], aps[rc][oc])
                    nc.sync.dma_start(
                        out=out[rsl, half * 1024 : half * 1024 + 1024], in_=ost
                    )

            p3_alloc(0)
            p3_mms(0, (0, 1, 2))
            p3_alloc(1)
            p3_mms(1, (0, 1, 2))
            p3_fin(0)
            p3_fin(1)
            p3_alloc(2)
            p3_mms(2, (0, 1, 2))
            p3_alloc(3)
            p3_mms(3, (0, 1, 2))
            p3_fin(2)
            p3_fin(3)
            att_pools.close()

    nc.compile()
    return nc



def _get_nc():
    global _NC
    if _NC is None:
        _NC = _build_nc()
    return _NC


def _make_in_maps(x, wq, bq, wk, bk, wv, bv, wo):
    qli = np.arange(128, dtype=np.int32)[None, :]
    kli = np.arange(128, dtype=np.int32)[:, None]
    tri_np = (qli >= kli).astype(NPBF16)

    in_maps = []
    for b in range(B):
        xTb = np.ascontiguousarray(np.asarray(x[b], dtype=np.float32).T).astype(NPBF16)
        for g in range(N_KV):
            in_maps.append(
                {
                    "xT": xTb,
                    "wq": np.ascontiguousarray(wq[:, g * 512 : (g + 1) * 512]).astype(
                        NPBF16
                    ),
                    "wk": np.ascontiguousarray(wk[:, g * 128 : (g + 1) * 128]).astype(
                        NPBF16
                    ),
                    "wv": np.ascontiguousarray(wv[:, g * 128 : (g + 1) * 128]).astype(
                        NPBF16
                    ),
                    "wo": np.ascontiguousarray(wo[g * 512 : (g + 1) * 512, :]).astype(
                        NPBF16
                    ),
                    "bq": np.asarray(bq[g * 512 : (g + 1) * 512], dtype=np.float32)
                    .reshape(512, 1)
                    .copy(),
                    "bk": np.asarray(bk[g * 128 : (g + 1) * 128], dtype=np.float32)
                    .reshape(128, 1)
                    .copy(),
                    "bv": np.asarray(bv[g * 128 : (g + 1) * 128], dtype=np.float32)
                    .reshape(128, 1)
                    .copy(),
                    "tri": tri_np,
                }
            )
    return in_maps


def run_device(x, wq, bq, wk, bk, wv, bv, wo, trace=False):
    """Run the SPMD kernel; returns (full_output, BassKernelResults)."""
    nc = _get_nc()
    in_maps = _make_in_maps(x, wq, bq, wk, bk, wv, bv, wo)
    res = run_bass_kernel_spmd(nc, in_maps, core_ids=list(range(8)), trace=trace)
    out = np.zeros((B, S, H), dtype=np.float32)
    for b in range(B):
        for g in range(N_KV):
            out[b] += res.results[b * N_KV + g]["out"].astype(np.float32)
    return out, res


def kernel(x, attention_mask, position_ids, wq, bq, wk, bk, wv, bv, wo):
    del attention_mask, position_ids  # causal mask is built on-chip
    x = np.asarray(x, dtype=np.float32)
    out, _ = run_device(
        x,
        np.asarray(wq, np.float32),
        np.asarray(bq, np.float32),
        np.asarray(wk, np.float32),
        np.asarray(bk, np.float32),
        np.asarray(wv, np.float32),
        np.asarray(bv, np.float32),
        np.asarray(wo, np.float32),
    )
    return out

